# revision 6
# baseline (speedup 1.0000x reference)
"""Group MoE layer (2 groups x 4 experts, top-1 group / top-2 expert routing)
on 8 Trainium2 NeuronCores via expert parallelism.

Strategy:
  - Host computes the (tiny) routing: language-gate argmax over groups,
    per-group expert top-k + softmax weights.
  - Tokens are dispatched by (group, expert) assignment: core c = g*4+e
    receives exactly the tokens routed to expert (g, e), padded to a common
    capacity C (SPMD: all cores run the same program).
  - Each core runs the dense FFN for its expert:
        Y^T = W2 @ relu(W1 @ X^T + b1) + b2      (tokens in the moving dim)
    with bf16 weights/activations and fp32 PSUM accumulation.
  - All device tensors are packed tile-contiguous on the host so every DMA
    is a handful of large descriptors (the DMA engines are descriptor-rate
    bound on [128, small] strided loads).
  - Host scatter-adds the weighted expert outputs back into the full output.
"""

import numpy as np
import ml_dtypes

import concourse.bacc as bacc
import concourse.mybir as mybir
from concourse import tile
from concourse import bass_utils

B, L, D, H = 2, 2048, 1024, 4096
G, E = 2, 4
NCORES = G * E
PART = 128
TOK_BLK = 512
HCH = 512                       # W1 h-chunk width (4 h-tiles)

_BF16 = ml_dtypes.bfloat16

_program_cache: dict[tuple, object] = {}


def _build(C: int, d: int = D, h: int = H, tok_blk: int = TOK_BLK, hch: int = HCH):
    """Build + compile the per-core expert FFN program for capacity C."""
    key = (C, d, h, tok_blk, hch)
    if key in _program_cache:
        return _program_cache[key]

    nd = d // PART
    nh = h // PART
    nhc = h // hch
    hpc = hch // PART              # h-tiles per chunk
    nblk = (C + tok_blk - 1) // tok_blk

    bf16 = mybir.dt.bfloat16
    f32 = mybir.dt.float32

    nc = bacc.Bacc("TRN2", target_bir_lowering=False, debug=False,
                   num_devices=NCORES)

    # Tile-contiguous packed layouts (one large DMA descriptor per tile).
    xt = nc.dram_tensor("xt", [nblk, nd, PART, tok_blk], bf16,
                        kind="ExternalInput")
    w1t = nc.dram_tensor("w1t", [nhc, nd, PART, hch], bf16,
                         kind="ExternalInput")
    w2t = nc.dram_tensor("w2t", [nh, PART, d], bf16, kind="ExternalInput")
    b1t = nc.dram_tensor("b1t", [PART, nh], f32, kind="ExternalInput")
    b2t = nc.dram_tensor("b2t", [PART, nd], f32, kind="ExternalInput")
    yt = nc.dram_tensor("yt", [nblk, nd, PART, tok_blk], f32,
                        kind="ExternalOutput")

    with tile.TileContext(nc) as tc:
        with (
            tc.tile_pool(name="wpool", bufs=1) as wpool,
            tc.tile_pool(name="h1pool", bufs=nh) as h1pool,
            tc.tile_pool(name="ypool", bufs=nd) as ypool,
            tc.tile_pool(name="ps1", bufs=4, space="PSUM") as ps1,
            tc.tile_pool(name="ps2", bufs=4, space="PSUM") as ps2,
        ):
            x_sb = [[None] * nblk for _ in range(nd)]
            w1_sb = [[None] * nhc for _ in range(nd)]

            def load_x(blk):
                for di in range(nd):
                    t = wpool.tile([PART, tok_blk], bf16, tag=f"x_{di}_{blk}")
                    nc.sync.dma_start(out=t[:, :], in_=xt.ap()[blk, di, :, :])
                    x_sb[di][blk] = t

            def load_w1(hc):
                for di in range(nd):
                    t = wpool.tile([PART, hch], bf16, tag=f"w1_{di}_{hc}")
                    nc.sync.dma_start(out=t[:, :], in_=w1t.ap()[hc, di, :, :])
                    w1_sb[di][hc] = t

            # Need-ordered loads: PE can start once x block 0 + W1 chunk 0 land.
            load_x(0)
            load_w1(0)
            for hc in range(1, nhc):
                load_w1(hc)
            for blk in range(1, nblk):
                load_x(blk)
            w2_sb = []
            for hi in range(nh):
                t = wpool.tile([PART, d], bf16, tag=f"w2_{hi}")
                nc.sync.dma_start(out=t[:, :], in_=w2t.ap()[hi, :, :])
                w2_sb.append(t)
            b1_sb = wpool.tile([PART, nh], f32, tag="b1")
            nc.sync.dma_start(out=b1_sb[:, :], in_=b1t.ap()[:, :])
            b2_sb = wpool.tile([PART, nd], f32, tag="b2")
            nc.sync.dma_start(out=b2_sb[:, :], in_=b2t.ap()[:, :])

            for blk in range(nblk):
                n0 = blk * tok_blk
                n = min(tok_blk, C - n0)
                h1_tiles = []
                for hi in range(nh):
                    hc, ho = divmod(hi, hpc)
                    ps = ps1.tile([PART, tok_blk], f32, tag="ps1")
                    for di in range(nd):
                        nc.tensor.matmul(
                            ps[:, :n],
                            w1_sb[di][hc][:, ho * PART:(ho + 1) * PART],
                            x_sb[di][blk][:, :n],
                            start=(di == 0), stop=(di == nd - 1),
                        )
                    h1 = h1pool.tile([PART, tok_blk], bf16, tag="h1")
                    nc.scalar.activation(h1[:, :n], ps[:, :n],
                                         mybir.ActivationFunctionType.Relu,
                                         bias=b1_sb[:, hi:hi + 1], scale=1.0)
                    h1_tiles.append(h1)
                for di in range(nd):
                    ps = ps2.tile([PART, tok_blk], f32, tag="ps2")
                    for hi in range(nh):
                        nc.tensor.matmul(
                            ps[:, :n],
                            w2_sb[hi][:, di * PART:(di + 1) * PART],
                            h1_tiles[hi][:, :n],
                            start=(hi == 0), stop=(hi == nh - 1),
                        )
                    y = ypool.tile([PART, tok_blk], f32, tag="y")
                    nc.vector.tensor_scalar_add(y[:, :n], ps[:, :n],
                                                b2_sb[:, di:di + 1])
                    nc.sync.dma_start(out=yt.ap()[blk, di, :, :n], in_=y[:, :n])

    nc.compile()
    _program_cache[key] = nc
    return nc


def _route(x, bn, Wlg, blg, Wg, k):
    """Numpy replica of the reference routing. Returns per-(g,e) assignment."""
    glog = bn @ Wlg.T + blg                       # (N, G)
    sel_group = np.argmax(glog, axis=1)           # (N,)
    assign = []
    for g in range(Wg.shape[0]):
        logits = x @ Wg[g].T                      # (N, E)
        order = np.argsort(-logits, axis=1, kind="stable")
        sel = order[:, :k]                        # (N, k)
        top = np.take_along_axis(logits, sel, axis=1).astype(np.float32)
        m = top.max(axis=1, keepdims=True)
        ex = np.exp(top - m)
        w = ex / ex.sum(axis=1, keepdims=True)    # (N, k)
        assign.append((sel, w))
    return sel_group, assign


def kernel(**inputs) -> np.ndarray:
    xs = np.asarray(inputs["xs"], np.float32)
    bn = np.asarray(inputs["bottle_neck"], np.float32)
    Wlg = np.asarray(inputs["Wlg"], np.float32)
    blg = np.asarray(inputs["blg"], np.float32)
    Wg = np.asarray(inputs["Wg"], np.float32)
    W1 = np.asarray(inputs["W1"], np.float32)
    b1 = np.asarray(inputs["b1"], np.float32)
    W2 = np.asarray(inputs["W2"], np.float32)
    b2 = np.asarray(inputs["b2"], np.float32)
    k = int(np.asarray(inputs["top_k"]))

    Bx, Lx, d = xs.shape
    hdim = W1.shape[2]
    N = Bx * Lx
    nd = d // PART
    nh = hdim // PART
    nhc = hdim // HCH
    x = xs.reshape(N, d)
    bnf = bn.reshape(N, d)

    sel_group, assign = _route(x, bnf, Wlg, blg, Wg, k)

    # Token sets per (group, expert) core.
    idxs, wgts = [], []
    for c in range(NCORES):
        g, e = divmod(c, E)
        sel, w = assign[g]
        mask = (sel_group == g)[:, None] & (sel == e)
        rows, cols = np.nonzero(mask)
        idxs.append(rows)
        wgts.append(w[rows, cols])

    cnt_max = max(len(i) for i in idxs)
    C = max(PART, -(-cnt_max // PART) * PART)     # pad capacity to 128
    nblk = (C + TOK_BLK - 1) // TOK_BLK

    nc = _build(C, d, hdim)

    in_maps = []
    for c in range(NCORES):
        g, e = divmod(c, E)
        cnt = len(idxs[c])
        # tokens^T, packed [nblk, nd, 128, TOK_BLK] tile-contiguous
        xt = np.zeros((nblk * TOK_BLK, d), np.float32)
        if cnt:
            xt[:cnt] = x[idxs[c]]
        xt = np.ascontiguousarray(
            xt.T.astype(_BF16)                     # (d, nblk*TOK_BLK)
              .reshape(nd, PART, nblk, TOK_BLK)
              .transpose(2, 0, 1, 3))
        w1p = np.ascontiguousarray(
            W1[g, e].T.astype(_BF16)               # (d, h)
                 .reshape(nd, PART, nhc, HCH)
                 .transpose(2, 0, 1, 3))           # [nhc, nd, 128, HCH]
        w2p = np.ascontiguousarray(
            W2[g, e].T.astype(_BF16)               # (h, d)
                 .reshape(nh, PART, d))            # [nh, 128, d]
        in_maps.append({
            "xt": xt,
            "w1t": w1p,
            "w2t": w2p,
            "b1t": np.ascontiguousarray(b1[g, e].reshape(nh, PART).T),
            "b2t": np.ascontiguousarray(b2[g, e].reshape(nd, PART).T),
        })

    res = bass_utils.run_bass_kernel_spmd(nc, in_maps, core_ids=list(range(NCORES)))

    out = np.zeros((N, d), np.float32)
    for c in range(NCORES):
        cnt = len(idxs[c])
        if cnt == 0:
            continue
        yt = res.results[c]["yt"]                  # [nblk, nd, 128, TOK_BLK]
        yc = yt.transpose(1, 2, 0, 3).reshape(d, nblk * TOK_BLK)[:, :cnt].T
        out[idxs[c]] += wgts[c][:, None] * yc
    return out.reshape(Bx, Lx, d).astype(np.float32)


# revision 8
# speedup vs baseline: 1.1048x; 1.1048x over previous
"""Group MoE layer (2 groups x 4 experts, top-1 group / top-2 expert routing)
on 8 Trainium2 NeuronCores via expert parallelism.

Strategy:
  - Host computes the (tiny) routing: language-gate argmax over groups,
    per-group expert top-k + softmax weights.
  - Tokens are dispatched by (group, expert) assignment: core c = g*4+e
    receives exactly the tokens routed to expert (g, e), padded to a common
    capacity C (SPMD: all cores run the same program).
  - Each core runs the dense FFN for its expert:
        Y^T = W2 @ relu(W1 @ X^T + b1) + b2      (tokens in the moving dim)
    with bf16 weights/activations and fp32 PSUM accumulation.
  - DMA triggers cost ~650ns each on the issuing sequencer (descriptor
    generation, one descriptor per partition row), so tensors are packed on
    the host such that each dma_start moves one big merged tile with 8-32KB
    contiguous per partition row: few triggers, large descriptors.
  - Host scatter-adds the weighted expert outputs back into the full output.
"""

import numpy as np
import ml_dtypes

import concourse.bacc as bacc
import concourse.mybir as mybir
from concourse import tile
from concourse import bass_utils

B, L, D, H = 2, 2048, 1024, 4096
G, E = 2, 4
NCORES = G * E
PART = 128
TOK_BLK = 512
W1CH = 1024                     # W1 h-chunk width (8 h-tiles per chunk)
W2GRP = 16                      # h-tiles per merged W2 tile

_BF16 = ml_dtypes.bfloat16

_program_cache: dict[tuple, object] = {}


def _build(C: int, d: int = D, h: int = H, tok_blk: int = TOK_BLK,
           w1ch: int = W1CH, w2grp: int = W2GRP):
    """Build + compile the per-core expert FFN program for capacity C."""
    key = (C, d, h, tok_blk, w1ch, w2grp)
    if key in _program_cache:
        return _program_cache[key]

    nd = d // PART
    nh = h // PART
    nhc = h // w1ch
    hpc = w1ch // PART             # h-tiles per W1 chunk
    ng2 = nh // w2grp              # merged W2 tiles
    nblk = (C + tok_blk - 1) // tok_blk

    bf16 = mybir.dt.bfloat16
    f32 = mybir.dt.float32

    nc = bacc.Bacc("TRN2", target_bir_lowering=False, debug=False,
                   num_devices=NCORES)

    # Merged-tile layouts: per partition row everything is contiguous, so
    # each dma_start is 128 large descriptors.
    xt = nc.dram_tensor("xt", [nblk, PART, nd * tok_blk], bf16,
                        kind="ExternalInput")
    w1t = nc.dram_tensor("w1t", [nhc, PART, nd * w1ch], bf16,
                         kind="ExternalInput")
    w2t = nc.dram_tensor("w2t", [ng2, PART, w2grp * d], bf16,
                         kind="ExternalInput")
    b1t = nc.dram_tensor("b1t", [PART, nh], f32, kind="ExternalInput")
    b2t = nc.dram_tensor("b2t", [PART, nd], f32, kind="ExternalInput")
    yt = nc.dram_tensor("yt", [nblk, PART, nd * tok_blk], f32,
                        kind="ExternalOutput")

    with tile.TileContext(nc) as tc:
        with (
            tc.tile_pool(name="wpool", bufs=1) as wpool,
            tc.tile_pool(name="h1pool", bufs=nh) as h1pool,
            tc.tile_pool(name="ypool", bufs=1) as ypool,
            tc.tile_pool(name="ps1", bufs=4, space="PSUM") as ps1,
            tc.tile_pool(name="ps2", bufs=4, space="PSUM") as ps2,
        ):
            x_sb = [None] * nblk
            w1_sb = [None] * nhc
            w2_sb = [None] * ng2

            def load_x(blk):
                t = wpool.tile([PART, nd * tok_blk], bf16, tag=f"x_{blk}")
                nc.sync.dma_start(out=t[:, :], in_=xt.ap()[blk])
                x_sb[blk] = t

            def load_w1(hc):
                t = wpool.tile([PART, nd * w1ch], bf16, tag=f"w1_{hc}")
                nc.sync.dma_start(out=t[:, :], in_=w1t.ap()[hc])
                w1_sb[hc] = t

            # Need-ordered loads: PE starts once x block 0 + W1 chunk 0 land.
            load_x(0)
            load_w1(0)
            if nhc > 1:
                load_w1(1)
            for blk in range(1, nblk):
                load_x(blk)
            for hc in range(2, nhc):
                load_w1(hc)
            for gi in range(ng2):
                t = wpool.tile([PART, w2grp * d], bf16, tag=f"w2_{gi}")
                nc.sync.dma_start(out=t[:, :], in_=w2t.ap()[gi])
                w2_sb[gi] = t
            b1_sb = wpool.tile([PART, nh], f32, tag="b1")
            nc.sync.dma_start(out=b1_sb[:, :], in_=b1t.ap()[:, :])
            b2_sb = wpool.tile([PART, nd], f32, tag="b2")
            nc.sync.dma_start(out=b2_sb[:, :], in_=b2t.ap()[:, :])

            for blk in range(nblk):
                n0 = blk * tok_blk
                n = min(tok_blk, C - n0)
                h1_tiles = []
                for hi in range(nh):
                    hc, ho = divmod(hi, hpc)
                    ps = ps1.tile([PART, tok_blk], f32, tag="ps1")
                    for di in range(nd):
                        nc.tensor.matmul(
                            ps[:, :n],
                            w1_sb[hc][:, di * w1ch + ho * PART:
                                      di * w1ch + (ho + 1) * PART],
                            x_sb[blk][:, di * tok_blk:di * tok_blk + n],
                            start=(di == 0), stop=(di == nd - 1),
                        )
                    h1 = h1pool.tile([PART, tok_blk], bf16, tag="h1")
                    nc.scalar.activation(h1[:, :n], ps[:, :n],
                                         mybir.ActivationFunctionType.Relu,
                                         bias=b1_sb[:, hi:hi + 1], scale=1.0)
                    h1_tiles.append(h1)
                y = ypool.tile([PART, nd * tok_blk], f32, tag="y")
                if n < tok_blk:
                    # partial block: zero the tile so the full-width DMA-out
                    # below never reads uninitialized SBUF
                    nc.vector.memset(y[:, :], 0.0)
                for di in range(nd):
                    ps = ps2.tile([PART, tok_blk], f32, tag="ps2")
                    for hi in range(nh):
                        gi, hj = divmod(hi, w2grp)
                        nc.tensor.matmul(
                            ps[:, :n],
                            w2_sb[gi][:, hj * d + di * PART:
                                      hj * d + (di + 1) * PART],
                            h1_tiles[hi][:, :n],
                            start=(hi == 0), stop=(hi == nh - 1),
                        )
                    nc.vector.tensor_scalar_add(
                        y[:, di * tok_blk:di * tok_blk + n], ps[:, :n],
                        b2_sb[:, di:di + 1])
                nc.sync.dma_start(out=yt.ap()[blk], in_=y[:, :])

    nc.compile()
    _program_cache[key] = nc
    return nc


def _route(x, bn, Wlg, blg, Wg, k):
    """Numpy replica of the reference routing. Returns per-(g,e) assignment."""
    glog = bn @ Wlg.T + blg                       # (N, G)
    sel_group = np.argmax(glog, axis=1)           # (N,)
    assign = []
    for g in range(Wg.shape[0]):
        logits = x @ Wg[g].T                      # (N, E)
        order = np.argsort(-logits, axis=1, kind="stable")
        sel = order[:, :k]                        # (N, k)
        top = np.take_along_axis(logits, sel, axis=1).astype(np.float32)
        m = top.max(axis=1, keepdims=True)
        ex = np.exp(top - m)
        w = ex / ex.sum(axis=1, keepdims=True)    # (N, k)
        assign.append((sel, w))
    return sel_group, assign


def _pack_x(X, d, nblk, tok_blk):
    """(C_pad, d) fp32 -> [nblk, 128, nd*tok_blk] bf16, merged-tile layout."""
    nd = d // PART
    xt = X.T.astype(_BF16)                        # (d, nblk*tok_blk)
    return np.ascontiguousarray(
        xt.reshape(nd, PART, nblk, tok_blk).transpose(2, 1, 0, 3)
          .reshape(nblk, PART, nd * tok_blk))


def _pack_w1(W1e, d, h, w1ch):
    nd, nhc = d // PART, h // w1ch
    w = W1e.T.astype(_BF16)                       # (d, h)
    return np.ascontiguousarray(
        w.reshape(nd, PART, nhc, w1ch).transpose(2, 1, 0, 3)
         .reshape(nhc, PART, nd * w1ch))


def _pack_w2(W2e, d, h, w2grp):
    ng2 = h // PART // w2grp
    w = W2e.T.astype(_BF16)                       # (h, d)
    return np.ascontiguousarray(
        w.reshape(ng2, w2grp, PART, d).transpose(0, 2, 1, 3)
         .reshape(ng2, PART, w2grp * d))


def _unpack_y(yt, d, nblk, tok_blk):
    """[nblk, 128, nd*tok_blk] f32 -> (d, nblk*tok_blk)."""
    nd = d // PART
    return (yt.reshape(nblk, PART, nd, tok_blk).transpose(2, 1, 0, 3)
              .reshape(d, nblk * tok_blk))


def kernel(**inputs) -> np.ndarray:
    xs = np.asarray(inputs["xs"], np.float32)
    bn = np.asarray(inputs["bottle_neck"], np.float32)
    Wlg = np.asarray(inputs["Wlg"], np.float32)
    blg = np.asarray(inputs["blg"], np.float32)
    Wg = np.asarray(inputs["Wg"], np.float32)
    W1 = np.asarray(inputs["W1"], np.float32)
    b1 = np.asarray(inputs["b1"], np.float32)
    W2 = np.asarray(inputs["W2"], np.float32)
    b2 = np.asarray(inputs["b2"], np.float32)
    k = int(np.asarray(inputs["top_k"]))

    Bx, Lx, d = xs.shape
    hdim = W1.shape[2]
    N = Bx * Lx
    nh = hdim // PART
    nd = d // PART
    x = xs.reshape(N, d)
    bnf = bn.reshape(N, d)

    sel_group, assign = _route(x, bnf, Wlg, blg, Wg, k)

    # Token sets per (group, expert) core.
    idxs, wgts = [], []
    for c in range(NCORES):
        g, e = divmod(c, E)
        sel, w = assign[g]
        mask = (sel_group == g)[:, None] & (sel == e)
        rows, cols = np.nonzero(mask)
        idxs.append(rows)
        wgts.append(w[rows, cols])

    cnt_max = max(len(i) for i in idxs)
    C = max(PART, -(-cnt_max // PART) * PART)     # pad capacity to 128
    nblk = (C + TOK_BLK - 1) // TOK_BLK

    nc = _build(C, d, hdim)

    in_maps = []
    for c in range(NCORES):
        g, e = divmod(c, E)
        cnt = len(idxs[c])
        X = np.zeros((nblk * TOK_BLK, d), np.float32)
        if cnt:
            X[:cnt] = x[idxs[c]]
        in_maps.append({
            "xt": _pack_x(X, d, nblk, TOK_BLK),
            "w1t": _pack_w1(W1[g, e], d, hdim, W1CH),
            "w2t": _pack_w2(W2[g, e], d, hdim, W2GRP),
            "b1t": np.ascontiguousarray(b1[g, e].reshape(nh, PART).T),
            "b2t": np.ascontiguousarray(b2[g, e].reshape(nd, PART).T),
        })

    res = bass_utils.run_bass_kernel_spmd(nc, in_maps, core_ids=list(range(NCORES)))

    out = np.zeros((N, d), np.float32)
    for c in range(NCORES):
        cnt = len(idxs[c])
        if cnt == 0:
            continue
        yc = _unpack_y(res.results[c]["yt"], d, nblk, TOK_BLK)[:, :cnt].T
        out[idxs[c]] += wgts[c][:, None] * yc
    return out.reshape(Bx, Lx, d).astype(np.float32)


# revision 10
# speedup vs baseline: 1.2623x; 1.1426x over previous
"""Group MoE layer (2 groups x 4 experts, top-1 group / top-2 expert routing)
on 8 Trainium2 NeuronCores via expert parallelism.

Strategy:
  - Host computes the (tiny) routing: language-gate argmax over groups,
    per-group expert top-k + softmax weights.
  - Tokens are dispatched by (group, expert) assignment: core c = g*4+e
    receives exactly the tokens routed to expert (g, e), padded to a common
    capacity C (SPMD: all cores run the same program).
  - Each core runs the dense FFN for its expert:
        Y^T = W2 @ relu(W1 @ X^T + b1) + b2      (tokens in the moving dim)
    with bf16 weights/activations and fp32 PSUM accumulation.
  - DMA triggers cost ~650ns each on the issuing sequencer (descriptor
    generation, one descriptor per partition row), so tensors are packed on
    the host such that each dma_start moves one big merged tile with 8-32KB
    contiguous per partition row: few triggers, large descriptors.
  - Host scatter-adds the weighted expert outputs back into the full output.
"""

import numpy as np
import ml_dtypes

import concourse.bacc as bacc
import concourse.mybir as mybir
from concourse import tile
from concourse import bass_utils

B, L, D, H = 2, 2048, 1024, 4096
G, E = 2, 4
NCORES = G * E
PART = 128
TOK_BLK = 512
W1CH = 1024                     # W1 h-chunk width (8 h-tiles per chunk)
W2GRP = 16                      # h-tiles per merged W2 tile

_BF16 = ml_dtypes.bfloat16

_program_cache: dict[tuple, object] = {}


def _build(C: int, d: int = D, h: int = H, tok_blk: int = TOK_BLK,
           w1ch: int = W1CH, w2grp: int = W2GRP):
    """Build + compile the per-core expert FFN program for capacity C."""
    key = (C, d, h, tok_blk, w1ch, w2grp)
    if key in _program_cache:
        return _program_cache[key]

    nd = d // PART
    nh = h // PART
    nhc = h // w1ch
    hpc = w1ch // PART             # h-tiles per W1 chunk
    ng2 = nh // w2grp              # merged W2 tiles
    nblk = (C + tok_blk - 1) // tok_blk

    bf16 = mybir.dt.bfloat16
    f32 = mybir.dt.float32

    nc = bacc.Bacc("TRN2", target_bir_lowering=False, debug=False,
                   num_devices=NCORES)

    # Merged-tile layouts: per partition row everything is contiguous, so
    # each dma_start is 128 large descriptors.
    xt = nc.dram_tensor("xt", [nblk, PART, nd * tok_blk], bf16,
                        kind="ExternalInput")
    w1t = nc.dram_tensor("w1t", [nhc, PART, nd * w1ch], bf16,
                         kind="ExternalInput")
    w2t = nc.dram_tensor("w2t", [ng2, PART, w2grp * d], bf16,
                         kind="ExternalInput")
    b1t = nc.dram_tensor("b1t", [PART, nh], f32, kind="ExternalInput")
    b2t = nc.dram_tensor("b2t", [PART, nd], f32, kind="ExternalInput")
    yt = nc.dram_tensor("yt", [nblk, PART, nd * tok_blk], f32,
                        kind="ExternalOutput")

    with tile.TileContext(nc) as tc:
        with (
            tc.tile_pool(name="wpool", bufs=1) as wpool,
            tc.tile_pool(name="h1pool", bufs=nh) as h1pool,
            tc.tile_pool(name="ypool", bufs=1) as ypool,
            tc.tile_pool(name="ps1", bufs=4, space="PSUM") as ps1,
            tc.tile_pool(name="ps2", bufs=4, space="PSUM") as ps2,
        ):
            x_sb = [None] * nblk
            w1_sb = [None] * nhc
            w2_sb = [None] * ng2

            def load_x(blk):
                t = wpool.tile([PART, nd * tok_blk], bf16, tag=f"x_{blk}")
                nc.sync.dma_start(out=t[:, :], in_=xt.ap()[blk])
                x_sb[blk] = t

            def load_w1(hc):
                t = wpool.tile([PART, nd * w1ch], bf16, tag=f"w1_{hc}")
                nc.sync.dma_start(out=t[:, :], in_=w1t.ap()[hc])
                w1_sb[hc] = t

            # Biases first (tiny, needed by the first relu), on the scalar
            # engine's HWDGE queue so they never queue behind bulk weights.
            b1_sb = wpool.tile([PART, nh], f32, tag="b1")
            nc.scalar.dma_start(out=b1_sb[:, :], in_=b1t.ap()[:, :])
            b2_sb = wpool.tile([PART, nd], f32, tag="b2")
            nc.scalar.dma_start(out=b2_sb[:, :], in_=b2t.ap()[:, :])
            # Need-ordered loads on the sync queue: PE starts once x block 0
            # + W1 chunk 0 land; W2 goes on the scalar queue in parallel.
            load_x(0)
            load_w1(0)
            for hc in range(1, nhc):
                load_w1(hc)
            for blk in range(1, nblk):
                load_x(blk)
            for gi in range(ng2):
                t = wpool.tile([PART, w2grp * d], bf16, tag=f"w2_{gi}")
                nc.scalar.dma_start(out=t[:, :], in_=w2t.ap()[gi])
                w2_sb[gi] = t

            for blk in range(nblk):
                n0 = blk * tok_blk
                n = min(tok_blk, C - n0)
                h1_tiles = []
                for hi in range(nh):
                    hc, ho = divmod(hi, hpc)
                    ps = ps1.tile([PART, tok_blk], f32, tag="ps1")
                    for di in range(nd):
                        nc.tensor.matmul(
                            ps[:, :n],
                            w1_sb[hc][:, di * w1ch + ho * PART:
                                      di * w1ch + (ho + 1) * PART],
                            x_sb[blk][:, di * tok_blk:di * tok_blk + n],
                            start=(di == 0), stop=(di == nd - 1),
                        )
                    h1 = h1pool.tile([PART, tok_blk], bf16, tag="h1")
                    nc.scalar.activation(h1[:, :n], ps[:, :n],
                                         mybir.ActivationFunctionType.Relu,
                                         bias=b1_sb[:, hi:hi + 1], scale=1.0)
                    h1_tiles.append(h1)
                y = ypool.tile([PART, nd * tok_blk], f32, tag="y")
                for di in range(nd):
                    ps = ps2.tile([PART, tok_blk], f32, tag="ps2")
                    for hi in range(nh):
                        gi, hj = divmod(hi, w2grp)
                        nc.tensor.matmul(
                            ps[:, :n],
                            w2_sb[gi][:, hj * d + di * PART:
                                      hj * d + (di + 1) * PART],
                            h1_tiles[hi][:, :n],
                            start=(hi == 0), stop=(hi == nh - 1),
                        )
                    nc.vector.tensor_scalar_add(
                        y[:, di * tok_blk:di * tok_blk + n], ps[:, :n],
                        b2_sb[:, di:di + 1])
                    # drain each d-tile as soon as it's ready (overlaps mm2,
                    # shrinks the end-of-kernel tail to one 256KB transfer)
                    nc.sync.dma_start(
                        out=yt.ap()[blk][:, di * tok_blk:di * tok_blk + n],
                        in_=y[:, di * tok_blk:di * tok_blk + n])

    nc.compile()
    _program_cache[key] = nc
    return nc


def _route(x, bn, Wlg, blg, Wg, k):
    """Numpy replica of the reference routing. Returns per-(g,e) assignment."""
    glog = bn @ Wlg.T + blg                       # (N, G)
    sel_group = np.argmax(glog, axis=1)           # (N,)
    assign = []
    for g in range(Wg.shape[0]):
        logits = x @ Wg[g].T                      # (N, E)
        order = np.argsort(-logits, axis=1, kind="stable")
        sel = order[:, :k]                        # (N, k)
        top = np.take_along_axis(logits, sel, axis=1).astype(np.float32)
        m = top.max(axis=1, keepdims=True)
        ex = np.exp(top - m)
        w = ex / ex.sum(axis=1, keepdims=True)    # (N, k)
        assign.append((sel, w))
    return sel_group, assign


def _pack_x(X, d, nblk, tok_blk):
    """(C_pad, d) fp32 -> [nblk, 128, nd*tok_blk] bf16, merged-tile layout."""
    nd = d // PART
    xt = X.T.astype(_BF16)                        # (d, nblk*tok_blk)
    return np.ascontiguousarray(
        xt.reshape(nd, PART, nblk, tok_blk).transpose(2, 1, 0, 3)
          .reshape(nblk, PART, nd * tok_blk))


def _pack_w1(W1e, d, h, w1ch):
    nd, nhc = d // PART, h // w1ch
    w = W1e.T.astype(_BF16)                       # (d, h)
    return np.ascontiguousarray(
        w.reshape(nd, PART, nhc, w1ch).transpose(2, 1, 0, 3)
         .reshape(nhc, PART, nd * w1ch))


def _pack_w2(W2e, d, h, w2grp):
    ng2 = h // PART // w2grp
    w = W2e.T.astype(_BF16)                       # (h, d)
    return np.ascontiguousarray(
        w.reshape(ng2, w2grp, PART, d).transpose(0, 2, 1, 3)
         .reshape(ng2, PART, w2grp * d))


def _unpack_y(yt, d, nblk, tok_blk):
    """[nblk, 128, nd*tok_blk] f32 -> (d, nblk*tok_blk)."""
    nd = d // PART
    return (yt.reshape(nblk, PART, nd, tok_blk).transpose(2, 1, 0, 3)
              .reshape(d, nblk * tok_blk))


def kernel(**inputs) -> np.ndarray:
    xs = np.asarray(inputs["xs"], np.float32)
    bn = np.asarray(inputs["bottle_neck"], np.float32)
    Wlg = np.asarray(inputs["Wlg"], np.float32)
    blg = np.asarray(inputs["blg"], np.float32)
    Wg = np.asarray(inputs["Wg"], np.float32)
    W1 = np.asarray(inputs["W1"], np.float32)
    b1 = np.asarray(inputs["b1"], np.float32)
    W2 = np.asarray(inputs["W2"], np.float32)
    b2 = np.asarray(inputs["b2"], np.float32)
    k = int(np.asarray(inputs["top_k"]))

    Bx, Lx, d = xs.shape
    hdim = W1.shape[2]
    N = Bx * Lx
    nh = hdim // PART
    nd = d // PART
    x = xs.reshape(N, d)
    bnf = bn.reshape(N, d)

    sel_group, assign = _route(x, bnf, Wlg, blg, Wg, k)

    # Token sets per (group, expert) core.
    idxs, wgts = [], []
    for c in range(NCORES):
        g, e = divmod(c, E)
        sel, w = assign[g]
        mask = (sel_group == g)[:, None] & (sel == e)
        rows, cols = np.nonzero(mask)
        idxs.append(rows)
        wgts.append(w[rows, cols])

    cnt_max = max(len(i) for i in idxs)
    C = max(PART, -(-cnt_max // PART) * PART)     # pad capacity to 128
    nblk = (C + TOK_BLK - 1) // TOK_BLK

    nc = _build(C, d, hdim)

    in_maps = []
    for c in range(NCORES):
        g, e = divmod(c, E)
        cnt = len(idxs[c])
        X = np.zeros((nblk * TOK_BLK, d), np.float32)
        if cnt:
            X[:cnt] = x[idxs[c]]
        in_maps.append({
            "xt": _pack_x(X, d, nblk, TOK_BLK),
            "w1t": _pack_w1(W1[g, e], d, hdim, W1CH),
            "w2t": _pack_w2(W2[g, e], d, hdim, W2GRP),
            "b1t": np.ascontiguousarray(b1[g, e].reshape(nh, PART).T),
            "b2t": np.ascontiguousarray(b2[g, e].reshape(nd, PART).T),
        })

    res = bass_utils.run_bass_kernel_spmd(nc, in_maps, core_ids=list(range(NCORES)))

    out = np.zeros((N, d), np.float32)
    for c in range(NCORES):
        cnt = len(idxs[c])
        if cnt == 0:
            continue
        yc = _unpack_y(res.results[c]["yt"], d, nblk, TOK_BLK)[:, :cnt].T
        out[idxs[c]] += wgts[c][:, None] * yc
    return out.reshape(Bx, Lx, d).astype(np.float32)
